# revision 2
# baseline (speedup 1.0000x reference)
"""GQA attention (B=1, S=2048, D=2048, 32 Q heads / 8 KV heads, RoPE, causal)
sharded tensor-parallel over KV-head groups across 8 NeuronCores.

v2 design:
- Host supplies x^T; q/k/v computed directly in transposed layout (no PE
  transposes of x, no q/k transposes).
- RoPE in transposed layout via host-side permutation of q/k head dims
  (pairs land 32 partitions apart); rotate-half is fused into the sin-muls
  with partition-offset operands (no shuffle copies). V rides the same code
  path with cos=1/sin=0 tables.
- V transposed back to natural per 128-chunk on the PE (16 tiny transposes).
- Scores for the two heads of a partition pair go into one 2-bank
  [128,1024] f32 PSUM tile -> one wide exp per chunk (strided view trims
  the causal diagonal).
- Engine pinning: ACT = exp only; DVE = psum-side elementwise; Pool =
  sbuf-only adds/masks.
- Output written bf16; host sums the 8 partials.

_build(reps=N) replicates the complete body N times (each rep re-loads all
inputs and rewrites the output) for steady-state slope timing.
"""

import sys

for _p in ("/opt/trn_rl_repo",):
    if _p not in sys.path:
        sys.path.insert(0, _p)

import ml_dtypes
import numpy as np

import concourse.bacc as bacc
import concourse.bass as bass
import concourse.mybir as mybir
import concourse.tile as tile
from concourse.bass_utils import run_bass_kernel_spmd
from concourse.masks import make_identity, make_upper_triangular

F32 = mybir.dt.float32
BF16 = mybir.dt.bfloat16

B, S, DIM = 1, 2048, 2048
NH, NKV, HD = 32, 8, 64
NHPC = NH // NKV          # q heads per core = 4
QSH = NHPC * HD           # q cols per core = 256
KVW = HD                  # kv cols per core = 64
QKVW = QSH + 2 * KVW      # fused qkv width = 384
NCORES = 8
P = 128
ND = DIM // P             # 16 d-chunks
NS = S // P               # 16 s-chunks of 128
ST = 512                  # s tile width for projections
NST = S // ST             # 4 s-tiles
SQT = 512                 # sq tile width for attention/wo
NJ = S // SQT             # 4 sq tiles
SCALE = HD ** -0.5
H2 = HD // 2              # 32


def _rep_body(tc, ctx, pools, tensors):
    nc = tc.nc
    (xT, wqkvt, wot, cosq, sinq, cosk, sink, out) = tensors
    (consts, ps_sc, ps_pv, ps_misc, rope_pool, et_pool, rc_pool, ob_pool,
     os_pool) = pools

    # ---- consts (regenerated per rep: part of one execution's work) ----
    ident = consts.tile([P, P], BF16, name="ident")
    make_identity(nc, ident[:])
    m01 = consts.tile([P, P], F32, name="m01")  # m01[t, r] = 1 if r >= t else 0
    make_upper_triangular(nc, m01[:], val=1.0, diag=True)
    onesp = consts.tile([P, HD], BF16, name="onesp")
    nc.gpsimd.memset(onesp[:], 1.0)
    vones = consts.tile([P, NS * (HD + 1)], BF16, name="vones")
    nc.gpsimd.memset(vones[:], 1.0)

    # ---- resident inputs (spread across DMA queues: sync carries the
    # critical wq/xT stream; tables on gpsimd's queue land in parallel
    # before the first rope; wot on vector's queue, needed only by wo) ----
    wq_sb = consts.tile([P, ND * QKVW], BF16, name="wq_sb")
    nc.sync.dma_start(
        out=wq_sb[:].rearrange("p (c q) -> p c q", c=ND),
        in_=wqkvt[:].rearrange("(c p) q -> p c q", p=P),
    )
    cq = consts.tile([P, S], F32, name="cosq")
    sq_ = consts.tile([P, S], F32, name="sinq")
    ck = consts.tile([P, S], F32, name="cosk")
    sk = consts.tile([P, S], F32, name="sink")
    nc.gpsimd.dma_start(out=cq[:], in_=cosq[:])
    nc.gpsimd.dma_start(out=sq_[:], in_=sinq[:])
    nc.gpsimd.dma_start(out=ck[:], in_=cosk[:])
    nc.gpsimd.dma_start(out=sk[:], in_=sink[:])
    xt_sb = consts.tile([P, ND * S], BF16, name="xt_sb")  # [d-part, (dchunk, s)]
    for st in range(NST):
        nc.sync.dma_start(
            out=xt_sb[:].rearrange("p (c s) -> p c s", c=ND)[
                :, :, ST * st : ST * (st + 1)
            ],
            in_=xT[:].rearrange("(c p) s -> p c s", p=P)[
                :, :, ST * st : ST * (st + 1)
            ],
        )
    wot_sb0 = consts.tile([P, DIM], BF16, name="wot0")
    wot_sb1 = consts.tile([P, DIM], BF16, name="wot1")
    nc.scalar.dma_start(out=wot_sb0[:], in_=wot[0:P, :])
    nc.scalar.dma_start(out=wot_sb1[:], in_=wot[P : 2 * P, :])

    # ---- resident activations (transposed layouts) ----
    qt01 = consts.tile([P, S], BF16, name="qt01")  # heads 0,1 on partition halves
    qt23 = consts.tile([P, S], BF16, name="qt23")  # heads 2,3
    kvt = consts.tile([P, S], BF16, name="kvt")    # k on 0:64, vT on 64:128
    ktop = consts.tile([P, S], BF16, name="ktop")  # k duplicated on 64:128

    def proj_tile(f, st):
        """Project f-chunk f (0,1=q; 2=k|v) for s-slice st, rope, store."""
        sl = slice(ST * st, ST * (st + 1))
        pp = ps_sc.tile([P, ST], F32, tag="sc")
        for d in range(ND):
            nc.tensor.matmul(
                pp[:],
                wq_sb[:, d * QKVW + f * P : d * QKVW + f * P + P],
                xt_sb[:, d * S + ST * st : d * S + ST * (st + 1)],
                start=(d == 0), stop=(d == ND - 1),
            )
        cosT, sinT = (cq, sq_) if f < 2 else (ck, sk)
        # rotate-half fused into the sin-muls: rope pairs are (p, p^32)
        # within each 64-partition head; for the v half (f==2, 64:128) the
        # sink table is 0 so shuf contributes nothing.
        shuf = rope_pool.tile([P, ST], F32, tag="shuf")
        for q in range(4):
            src = 32 * (q ^ 1)
            nc.vector.tensor_mul(
                shuf[32 * q : 32 * q + 32, :],
                pp[src : src + 32, :],
                sinT[32 * q : 32 * q + 32, sl],
            )
        tmp = rope_pool.tile([P, ST], F32, tag="tmp")
        nc.vector.tensor_mul(tmp[:], pp[:], cosT[:, sl])
        dst = (qt01, qt23, kvt)[f]
        nc.gpsimd.tensor_add(dst[:, sl], tmp[:], shuf[:])
        if f == 2:
            # duplicate k on ktop's upper half for odd heads
            nc.gpsimd.tensor_copy(ktop[HD:P, sl], dst[0:HD, sl])
            # transpose vT back to natural v, append into vones
            for cc in range(ST // P):
                c = st * (ST // P) + cc
                tp = ps_misc.tile([P, HD], BF16, tag="misc")
                nc.tensor.matmul(
                    tp[:], dst[HD:P, P * c : P * (c + 1)], ident[HD:P, HD:P],
                    is_transpose=True,
                )
                nc.vector.tensor_copy(
                    vones[:, c * (HD + 1) : c * (HD + 1) + HD], tp[:]
                )

    def attn_tile(j):
        """Scores+softmax+PV+wo for sq tile j over head pairs."""
        ncv = (SQT // P) * (j + 1)
        osb01 = ob_pool.tile([P, SQT], BF16, tag="ob")
        osb23 = ob_pool.tile([P, SQT], BF16, tag="ob")
        for hp in range(2):  # head pair (2hp, 2hp+1) on partition halves
            qt = qt01 if hp == 0 else qt23
            ovpA = ps_pv.tile([HD + 1, SQT], F32, tag="pv")
            ovpB = ps_pv.tile([HD + 1, SQT], F32, tag="pv")
            ets = {}
            # software pipeline: scores+exp for chunk c, PV for chunk c-LAG
            LAG = 2
            for c in range(ncv + LAG):
                if c < ncv:
                    c0 = max(0, P * c - SQT * j)
                    sp = ps_sc.tile([P, 2 * SQT], F32, tag="sc")
                    nc.tensor.matmul(
                        sp[:, c0:SQT],
                        kvt[0:HD, P * c : P * (c + 1)],
                        qt[0:HD, SQT * j + c0 : SQT * (j + 1)],
                    )
                    nc.tensor.matmul(
                        sp[:, SQT + c0 : 2 * SQT],
                        ktop[HD:P, P * c : P * (c + 1)],
                        qt[HD:P, SQT * j + c0 : SQT * (j + 1)],
                    )
                    et = et_pool.tile([P, 2 * SQT], BF16, tag="et")
                    spv = sp[:].rearrange("p (h s) -> p h s", h=2)
                    etv = et[:].rearrange("p (h s) -> p h s", h=2)
                    nc.scalar.activation(
                        etv[:, :, c0:SQT], spv[:, :, c0:SQT],
                        mybir.ActivationFunctionType.Exp, scale=SCALE,
                    )
                    if c0 or P * c == SQT * j:  # diagonal chunk: triangle mask
                        nc.gpsimd.tensor_mul(
                            et[:, c0 : c0 + P], et[:, c0 : c0 + P], m01[:]
                        )
                        nc.gpsimd.tensor_mul(
                            et[:, SQT + c0 : SQT + c0 + P],
                            et[:, SQT + c0 : SQT + c0 + P], m01[:],
                        )
                    ets[c] = (et, c0)
                if c >= LAG:
                    pc = c - LAG
                    pet, pc0 = ets.pop(pc)
                    nc.tensor.matmul(
                        ovpA[:, pc0:SQT],
                        vones[:, pc * (HD + 1) : (pc + 1) * (HD + 1)],
                        pet[:, pc0:SQT],
                        start=(pc == 0), stop=(pc == ncv - 1),
                    )
                    nc.tensor.matmul(
                        ovpB[:, pc0:SQT],
                        vones[:, pc * (HD + 1) : (pc + 1) * (HD + 1)],
                        pet[:, SQT + pc0 : 2 * SQT],
                        start=(pc == 0), stop=(pc == ncv - 1),
                    )
            for sub, ovp in ((0, ovpA), (1, ovpB)):
                rc = rc_pool.tile([P, SQT], BF16, tag="rc")
                nc.vector.reciprocal(rc[HD : HD + 1, :], ovp[HD : HD + 1, :])
                rp = ps_misc.tile([HD, SQT], F32, tag="misc")
                nc.tensor.matmul(
                    rp[:], onesp[HD : HD + 1, 0:HD], rc[HD : HD + 1, :],
                    tile_position=(HD, 0),
                )
                dst = osb01 if hp == 0 else osb23
                lo = HD * sub
                nc.scalar.copy(dst[lo : lo + HD, :], ovp[0:HD, :])
                nc.vector.tensor_mul(
                    dst[lo : lo + HD, :], dst[lo : lo + HD, :], rp[:]
                )

        for m in range(SQT // P):
            for e in range(DIM // SQT):
                wp = ps_misc.tile([P, SQT], F32, tag="misc")
                nc.tensor.matmul(
                    wp[:], osb01[:, P * m : P * (m + 1)],
                    wot_sb0[:, SQT * e : SQT * (e + 1)],
                    start=True, stop=False,
                )
                nc.tensor.matmul(
                    wp[:], osb23[:, P * m : P * (m + 1)],
                    wot_sb1[:, SQT * e : SQT * (e + 1)],
                    start=False, stop=True,
                )
                ob = os_pool.tile([P, SQT], BF16, tag="os")
                if (m * (DIM // SQT) + e) % 8 < 3:
                    nc.scalar.copy(ob[:], wp[:])
                else:
                    nc.vector.tensor_copy(ob[:], wp[:])
                nc.sync.dma_start(
                    out=out[SQT * j + P * m : SQT * j + P * (m + 1),
                            SQT * e : SQT * (e + 1)],
                    in_=ob[:],
                )

    for st in range(NST):
        for f in range(3):
            proj_tile(f, st)
    for j in range(NJ):
        attn_tile(j)


def _body(tc, ctx, reps=1, barrier=False):
    nc = tc.nc
    xT = nc.dram_tensor("xT", [DIM, S], BF16, kind="ExternalInput")
    wqkvt = nc.dram_tensor("wqkvt", [DIM, QKVW], BF16, kind="ExternalInput")
    wot = nc.dram_tensor("wot", [QSH, DIM], BF16, kind="ExternalInput")
    cosq = nc.dram_tensor("cosq", [P, S], F32, kind="ExternalInput")
    sinq = nc.dram_tensor("sinq", [P, S], F32, kind="ExternalInput")
    cosk = nc.dram_tensor("cosk", [P, S], F32, kind="ExternalInput")
    sink = nc.dram_tensor("sink", [P, S], F32, kind="ExternalInput")
    out = nc.dram_tensor("out", [S, DIM], BF16, kind="ExternalOutput")
    tensors = (xT, wqkvt, wot, cosq, sinq, cosk, sink, out)

    consts = ctx.enter_context(tc.tile_pool(name="consts", bufs=1))
    # PSUM budget (8 banks): sc 2x2 + pv 2x1 + misc 2x1 = 8
    ps_sc = ctx.enter_context(tc.tile_pool(name="ps_sc", bufs=2, space="PSUM"))
    ps_pv = ctx.enter_context(tc.tile_pool(name="ps_pv", bufs=2, space="PSUM"))
    ps_misc = ctx.enter_context(
        tc.tile_pool(name="ps_misc", bufs=2, space="PSUM")
    )
    rope_pool = ctx.enter_context(tc.tile_pool(name="rope", bufs=4))
    et_pool = ctx.enter_context(tc.tile_pool(name="et", bufs=4))
    rc_pool = ctx.enter_context(tc.tile_pool(name="rc", bufs=4))
    ob_pool = ctx.enter_context(tc.tile_pool(name="ob", bufs=4))
    os_pool = ctx.enter_context(tc.tile_pool(name="os", bufs=4))
    pools = (consts, ps_sc, ps_pv, ps_misc, rope_pool, et_pool, rc_pool,
             ob_pool, os_pool)

    for r in range(reps):
        if r and barrier:
            nc.all_engine_barrier()
        _rep_body(tc, ctx, pools, tensors)


_CACHE = {}


def _build(reps=1, barrier=False):
    key = (reps, barrier)
    if key not in _CACHE:
        from contextlib import ExitStack

        nc = bacc.Bacc(None, target_bir_lowering=False)
        with tile.TileContext(nc) as tc, ExitStack() as ctx:
            with nc.allow_low_precision(reason="bf16 matmul pipeline"):
                _body(tc, ctx, reps=reps, barrier=barrier)
        nc.compile()
        _CACHE[key] = nc
    return _CACHE[key]


_PERM = np.concatenate([np.arange(0, HD, 2), np.arange(1, HD, 2)])  # evens|odds


def _host_tables(freqs_cis):
    """cos/sin tables [128, S] f32 in permuted-transposed layout.

    Partition p (within each 64-partition head block): p<32 -> original even
    dim 2p (freq p, sign -sin), 32<=p<64 -> odd dim 2(p-32)+1 (freq p-32,
    sign +sin). cosk/sink rows 64:128 are identity (cos=1, sin=0) so the
    same rope code passes v through untouched.
    """
    cos = np.asarray(freqs_cis)[..., 0].astype(np.float32).T  # (32, S)
    sin = np.asarray(freqs_cis)[..., 1].astype(np.float32).T
    c64 = np.concatenate([cos, cos], axis=0)                  # (64, S)
    s64 = np.concatenate([-sin, sin], axis=0)
    cosq = np.ascontiguousarray(np.tile(c64, (2, 1)))         # (128, S)
    sinq = np.ascontiguousarray(np.tile(s64, (2, 1)))
    cosk = np.ascontiguousarray(
        np.concatenate([c64, np.ones_like(c64)], axis=0)
    )
    sink = np.ascontiguousarray(
        np.concatenate([s64, np.zeros_like(s64)], axis=0)
    )
    return cosq, sinq, cosk, sink


def _host_inputs(x, wq, wk, wv, wo, freqs_cis):
    """Build the 8 per-core input maps (shared host prep done once)."""
    bf = ml_dtypes.bfloat16
    xT = np.ascontiguousarray(
        np.asarray(x, np.float32)[0].T.astype(bf)
    )  # (DIM, S)
    cosq, sinq, cosk, sink = _host_tables(freqs_cis)
    wq_f = np.asarray(wq, np.float32)
    wk_f = np.asarray(wk, np.float32)
    wv_f = np.asarray(wv, np.float32)
    wo_f = np.asarray(wo, np.float32)
    in_maps = []
    for c in range(NCORES):
        wq_c = wq_f[c * QSH : (c + 1) * QSH].reshape(NHPC, HD, DIM)[
            :, _PERM
        ].reshape(QSH, DIM)
        wk_c = wk_f[c * KVW : (c + 1) * KVW][_PERM]
        wv_c = wv_f[c * KVW : (c + 1) * KVW]
        wqkvt = np.ascontiguousarray(
            np.concatenate([wq_c, wk_c, wv_c], axis=0).T.astype(bf)  # (D, 384)
        )
        wot = np.ascontiguousarray(
            wo_f[:, c * QSH : (c + 1) * QSH].T.astype(bf)
        )
        in_maps.append(
            dict(xT=xT, wqkvt=wqkvt, wot=wot,
                 cosq=cosq, sinq=sinq, cosk=cosk, sink=sink)
        )
    return in_maps


def kernel(x, wq, wk, wv, wo, freqs_cis, mask):
    nc = _build(reps=1)
    in_maps = _host_inputs(x, wq, wk, wv, wo, freqs_cis)
    res = run_bass_kernel_spmd(nc, in_maps, list(range(NCORES)))
    _CACHE["last"] = res
    total = res.results[0]["out"].astype(np.float32)
    for c in range(1, NCORES):
        total = total + res.results[c]["out"].astype(np.float32)
    return total.reshape(B, S, DIM)


if __name__ == "__main__":
    _build()
    print("build ok")
